# revision 1
# baseline (speedup 1.0000x reference)
"""Trainium2 Bass kernel for CustomConv2d:
x[32,128,112,112] (f32) * weight[256,128,3,3] + bias[256], stride=1, pad=1
-> out[32,256,112,112] (f32).

Strategy: data-parallel over batch (4 images per core on 8 cores). On each
core the conv is computed as 9 shift-accumulated fp32r matmuls per output
tile: contraction dim = C_IN = 128 (exactly the PE array), weights
stationary [c_in=128, c_out_half=128], moving operand = 4 output rows
(448 spatial positions) of the input image resident in SBUF. The image is
stored zero-padded to 114x114 per partition so every tap is a full-size
matmul whose APs satisfy the fp32r ISA restrictions (even innermost
counts, even outer steps). Weights are pre-transposed on the host to
[c_in, kh, kw, c_out] so every weight DMA and matmul slice is contiguous.

fp32r runs the PE at full rate (1 cycle/row for N>=256, vs 4 for fp32)
with ~1.4e-4 relative error vs the fp32 reference. Per-core roofline:
2016 matmuls x 448 cycles at 2.4 GHz = 376 us compute vs ~235 us DMA
(28 MB in + 51 MB out at ~330 GB/s) -> compute-bound. Measured steady
state ~455 us/pass on TRN2 (in-NEFF loop slope), ~83% of fp32r peak.
Tuned knobs: x DMA split into 28 pieces (prefetch grain), all 8 PSUM
banks in flight, 8 output staging buffers.
"""

import numpy as np

B, C_IN, H, W = 32, 128, 112, 112
C_OUT, KS = 256, 3
N_CORES = 8
B_SH = B // N_CORES  # images per core
R = 4                # output rows per PSUM tile -> N = R*W = 448 <= 512
HP, WP = H + 2, W + 2  # padded image dims (114 x 114)

_cache = {}


def _build(psum_bufs=8, o_bufs=8, x_bufs=2, with_load=True, with_compute=True,
           with_store=True, x_split=28, taps_subset=None, loop_n=1,
           chunk_group=1, loop_stagger=False, store_batch=1):
    import contextlib

    import concourse.mybir as mybir
    from concourse import bacc
    from concourse.tile import TileContext

    f32 = mybir.dt.float32
    f32r = mybir.dt.float32r

    nc = bacc.Bacc("TRN2")
    x_d = nc.dram_tensor("x", [B_SH, C_IN, HP, WP], f32r, kind="ExternalInput")
    w_d = nc.dram_tensor("w", [C_IN, KS * KS * C_OUT], f32r, kind="ExternalInput")
    b_d = nc.dram_tensor("bias", [128, C_OUT // 128], f32, kind="ExternalInput")
    out_d = nc.dram_tensor("out", [B_SH, C_OUT, H, W], f32, kind="ExternalOutput")

    n_half = C_OUT // 128  # 2
    taps = [(dh, dw) for dh in (-1, 0, 1) for dw in (-1, 0, 1)]

    with TileContext(nc) as tc:
        with (
            tc.tile_pool(name="wpool", bufs=1) as wpool,
            tc.tile_pool(name="xpool", bufs=x_bufs) as xpool,
            tc.tile_pool(name="opool", bufs=o_bufs) as opool,
            tc.tile_pool(name="psum", bufs=psum_bufs, space="PSUM") as psum_pool,
        ):
            w_sb = wpool.tile([128, KS * KS * C_OUT], f32r)
            nc.sync.dma_start(out=w_sb[:], in_=w_d[:])
            bias_sb = wpool.tile([128, n_half], f32)
            nc.sync.dma_start(out=bias_sb[:], in_=b_d[:])

            loop_cm = (
                tc.For_i(0, loop_n, 1, staggered_reset=loop_stagger)
                if loop_n > 1
                else contextlib.nullcontext()
            )
            with loop_cm:
              for b in range(B_SH):
                x_sb = xpool.tile([128, HP * WP], f32r)
                x3 = x_sb[:].rearrange("c (h w) -> c h w", w=WP)
                # image arrives pre-padded (114x114 with zero border) from
                # the host, so this is one fully contiguous DMA (optionally
                # split into x_split pieces for finer prefetch overlap).
                if with_load:
                    xd_flat = x_d[b].rearrange("c h w -> c (h w)")
                    step = HP * WP // x_split
                    for s in range(x_split):
                        lo = s * step
                        hi = HP * WP if s == x_split - 1 else (s + 1) * step
                        nc.sync.dma_start(
                            out=x_sb[:, lo:hi], in_=xd_flat[:, lo:hi]
                        )
                use_taps = taps if taps_subset is None else taps[:taps_subset]
                G = chunk_group
                SB = store_batch
                assert (H // R) % (SB * G) == 0 or SB == 1
                osb = {}
                for h0 in range(0, H, R * G):
                    # G row-chunks x n_half psum banks in flight; tap-inner
                    # ordering reuses each stationary weight tile G times.
                    pss = {}
                    for g in range(G):
                        for m in range(n_half):
                            pss[(g, m)] = psum_pool.tile(
                                [128, R * W], f32, name="ps", tag="ps"
                            )
                    if with_compute:
                        for m in range(n_half):
                            for i, (dh, dw) in enumerate(use_taps):
                                t = (dh + 1) * KS + (dw + 1)
                                co = t * C_OUT + m * 128
                                for g in range(G):
                                    hg = h0 + g * R
                                    nc.tensor.matmul(
                                        pss[(g, m)][:].rearrange(
                                            "p (r w) -> p r w", w=W
                                        ),
                                        w_sb[:, co : co + 128],
                                        x3[:, hg + dh + 1 : hg + dh + 1 + R,
                                           dw + 1 : dw + 1 + W],
                                        start=(i == 0),
                                        stop=(i == len(use_taps) - 1),
                                    )
                    if with_store:
                        for g in range(G):
                            for m in range(n_half):
                                hg = h0 + g * R
                                ps = pss[(g, m)]
                                j = (hg // R) % SB
                                if j == 0:
                                    osb[m] = opool.tile(
                                        [128, SB * R * W], f32,
                                        name="osb", tag="osb",
                                    )
                                dst = osb[m][:, j * R * W : (j + 1) * R * W]
                                if with_compute:
                                    nc.vector.tensor_scalar_add(
                                        dst, ps[:], bias_sb[:, m : m + 1]
                                    )
                                else:
                                    nc.vector.tensor_scalar_add(
                                        dst,
                                        x_sb[:, : R * W].bitcast(f32),
                                        bias_sb[:, m : m + 1],
                                    )
                                if j == SB - 1:
                                    nc.sync.dma_start(
                                        out=out_d[
                                            b, m * 128 : (m + 1) * 128,
                                            hg - (SB - 1) * R : hg + R, :,
                                        ],
                                        in_=osb[m][:],
                                    )
    nc.finalize()
    return nc


def _get_nc():
    if "nc" not in _cache:
        _cache["nc"] = _build()
    return _cache["nc"]


def _make_fast_runner(nc):
    """Cached jitted shard_map callable (axon/PJRT only). Avoids the ~20s
    per-call retrace that run_bass_kernel_spmd pays. Returns
    run(in_maps) -> full-batch output array."""
    import jax
    import numpy as _np
    from jax.sharding import Mesh, NamedSharding, PartitionSpec
    from jax.experimental.shard_map import shard_map
    import concourse.mybir as mybir
    from concourse import bass2jax as b2j

    b2j.install_neuronx_cc_hook()
    partition_name = nc.partition_id_tensor.name if nc.partition_id_tensor else None
    in_names, out_names, out_avals = [], [], []
    for alloc in nc.m.functions[0].allocations:
        if not isinstance(alloc, mybir.MemoryLocationSet):
            continue
        name = alloc.memorylocations[0].name
        if alloc.kind == "ExternalInput":
            if name != partition_name:
                in_names.append(name)
        elif alloc.kind == "ExternalOutput":
            out_names.append(name)
            out_avals.append(
                jax.core.ShapedArray(
                    tuple(alloc.tensor_shape), mybir.dt.np(alloc.dtype)
                )
            )
    assert out_names == ["out"]
    n_params = len(in_names)
    all_in_names = list(in_names) + list(out_names)
    if partition_name is not None:
        all_in_names.append(partition_name)

    def _body(*args):
        operands = list(args)
        if partition_name is not None:
            operands.append(b2j.partition_id_tensor())
        return tuple(
            b2j._bass_exec_p.bind(
                *operands,
                out_avals=tuple(out_avals),
                in_names=tuple(all_in_names),
                out_names=tuple(out_names),
                lowering_input_output_aliases=(),
                sim_require_finite=True,
                sim_require_nnan=True,
                nc=nc,
            )
        )

    devices = jax.devices()[:N_CORES]
    assert len(devices) == N_CORES
    mesh = Mesh(_np.asarray(devices), ("core",))
    n_outs = len(out_avals)
    fn = jax.jit(
        shard_map(
            _body,
            mesh=mesh,
            in_specs=(PartitionSpec("core"),) * (n_params + n_outs),
            out_specs=(PartitionSpec("core"),) * n_outs,
            check_rep=False,
        ),
        keep_unused=True,
    )
    shard = NamedSharding(mesh, PartitionSpec("core"))
    zeros = [
        jax.device_put(
            _np.zeros((N_CORES * av.shape[0], *av.shape[1:]), av.dtype), shard
        )
        for av in out_avals
    ]

    def run(in_maps):
        ins = [
            jax.device_put(
                _np.concatenate([m[name] for m in in_maps], axis=0), shard
            )
            for name in in_names
        ]
        outs = fn(*ins, *zeros)
        return _np.asarray(outs[0]).reshape(B, C_OUT, H, W)

    return run


def _install_neff_cache():
    """Content-addressed NEFF cache keyed on the BIR bytes. The bass_exec
    compile path (bass2jax.neuronx_cc_hook -> compile_bir_kernel) has no
    caching, so every fresh process pays the full walrus compile (30s-6min)
    for a byte-identical NEFF. The Bass build is deterministic, so caching
    on sha256(ant_bir) is sound. Best effort: any failure falls through to
    the original compile."""
    try:
        import hashlib
        import pathlib
        import shutil

        from concourse import bass2jax as b2j

        if getattr(b2j, "_ant_neff_cache_installed", False):
            return
        orig = b2j.compile_bir_kernel
        cache_dir = pathlib.Path.home() / ".cache" / "bass_neff_cache"
        cache_dir.mkdir(parents=True, exist_ok=True)

        def cached(ant_bir_str, compile_dir_path, neff_name="file.neff", **kw):
            import os

            try:
                raw = (
                    ant_bir_str
                    if isinstance(ant_bir_str, bytes)
                    else str(ant_bir_str).encode()
                )
                key = hashlib.sha256(raw).hexdigest()
                cpath = cache_dir / f"{key}_{neff_name}"
                if cpath.exists():
                    out = os.path.join(compile_dir_path, neff_name)
                    shutil.copyfile(cpath, out)
                    return out
            except Exception:
                cpath = None
            res = orig(ant_bir_str, compile_dir_path, neff_name=neff_name, **kw)
            if cpath is not None:
                try:
                    tmp = str(cpath) + ".tmp"
                    shutil.copyfile(res, tmp)
                    os.replace(tmp, cpath)
                except Exception:
                    pass
            return res

        b2j.compile_bir_kernel = cached
        b2j._ant_neff_cache_installed = True
    except Exception:
        pass


def _run_spmd(in_maps):
    from concourse.bass_utils import run_bass_kernel_spmd

    res = run_bass_kernel_spmd(_get_nc(), in_maps, core_ids=list(range(N_CORES)))
    return np.concatenate(
        [res.results[c]["out"] for c in range(N_CORES)], axis=0
    )


def _run(in_maps):
    if "runner" not in _cache:
        _install_neff_cache()
        runner = None
        try:
            from concourse._compat import axon_active

            if axon_active():
                runner = _make_fast_runner(_get_nc())
        except Exception:
            runner = None
        _cache["runner"] = runner if runner is not None else _run_spmd
    try:
        return _cache["runner"](in_maps)
    except Exception:
        if _cache["runner"] is _run_spmd:
            raise
        # fast path failed at call time: fall back permanently
        _cache["runner"] = _run_spmd
        return _run_spmd(in_maps)


def kernel(x, weight, bias, stride=1, padding=1, **_ignored):
    assert int(stride) == 1 and int(padding) == 1
    x = np.asarray(x, dtype=np.float32)
    weight = np.asarray(weight, dtype=np.float32)
    bias = np.asarray(bias, dtype=np.float32)
    assert x.shape == (B, C_IN, H, W) and weight.shape == (C_OUT, C_IN, KS, KS)
    xp = np.pad(x, ((0, 0), (0, 0), (1, 1), (1, 1)))

    # [c_out, c_in, kh, kw] -> [c_in, kh, kw, c_out] so that the lhsT slice
    # for (tap, half) is contiguous along c_out with c_in on partitions.
    w_t = np.ascontiguousarray(np.transpose(weight, (1, 2, 3, 0))).reshape(
        C_IN, KS * KS * C_OUT
    )
    bias2 = np.ascontiguousarray(bias.reshape(C_OUT // 128, 128).T)

    in_maps = [
        {
            "x": np.ascontiguousarray(xp[c * B_SH : (c + 1) * B_SH]),
            "w": w_t,
            "bias": bias2,
        }
        for c in range(N_CORES)
    ]
    return _run(in_maps)



# revision 2
# speedup vs baseline: 45069.2104x; 45069.2104x over previous
"""Trainium2 Bass kernel for CustomConv2d:
x[32,128,112,112] (f32) * weight[256,128,3,3] + bias[256], stride=1, pad=1
-> out[32,256,112,112] (f32).

Strategy: data-parallel over batch (4 images per core on 8 cores). On each
core the conv is computed as 9 shift-accumulated fp32r matmuls per output
tile: contraction dim = C_IN = 128 (exactly the PE array), weights
stationary [c_in=128, c_out_half=128], moving operand = 4 output rows
(448 spatial positions) of the input image resident in SBUF. The image is
stored zero-padded to 114x114 per partition so every tap is a full-size
matmul whose APs satisfy the fp32r ISA restrictions (even innermost
counts, even outer steps). Weights are pre-transposed on the host to
[c_in, kh, kw, c_out] so every weight DMA and matmul slice is contiguous.

fp32r runs the PE at full rate (1 cycle/row for N>=256, vs 4 for fp32)
with ~1.4e-4 relative error vs the fp32 reference. Per-core roofline:
2016 matmuls x 448 cycles at 2.4 GHz = 376 us compute vs ~235 us DMA
(28 MB in + 51 MB out at ~330 GB/s) -> compute-bound. Measured steady
state ~455 us/pass on TRN2 (in-NEFF loop slope), ~83% of fp32r peak.
Tuned knobs: x DMA split into 28 pieces (prefetch grain), all 8 PSUM
banks in flight, 8 output staging buffers.
"""

import numpy as np

B, C_IN, H, W = 32, 128, 112, 112
C_OUT, KS = 256, 3
N_CORES = 8
B_SH = B // N_CORES  # images per core
R = 4                # output rows per PSUM tile -> N = R*W = 448 <= 512
HP, WP = H + 2, W + 2  # padded image dims (114 x 114)

_cache = {}


def _build(psum_bufs=8, o_bufs=8, x_bufs=2, with_load=True, with_compute=True,
           with_store=True, x_split=28, taps_subset=None, loop_n=1,
           chunk_group=1, loop_stagger=False, store_batch=1, dtype="f32r"):
    import contextlib

    import concourse.mybir as mybir
    from concourse import bacc
    from concourse.tile import TileContext

    f32 = mybir.dt.float32
    f32r = mybir.dt.float32r if dtype == "f32r" else mybir.dt.bfloat16

    nc = bacc.Bacc("TRN2")
    x_d = nc.dram_tensor("x", [B_SH, C_IN, HP, WP], f32r, kind="ExternalInput")
    w_d = nc.dram_tensor("w", [C_IN, KS * KS * C_OUT], f32r, kind="ExternalInput")
    b_d = nc.dram_tensor("bias", [128, C_OUT // 128], f32, kind="ExternalInput")
    out_d = nc.dram_tensor("out", [B_SH, C_OUT, H, W], f32, kind="ExternalOutput")

    n_half = C_OUT // 128  # 2
    taps = [(dh, dw) for dh in (-1, 0, 1) for dw in (-1, 0, 1)]

    with TileContext(nc) as tc:
        with (
            tc.tile_pool(name="wpool", bufs=1) as wpool,
            tc.tile_pool(name="xpool", bufs=x_bufs) as xpool,
            tc.tile_pool(name="opool", bufs=o_bufs) as opool,
            tc.tile_pool(name="psum", bufs=psum_bufs, space="PSUM") as psum_pool,
        ):
            w_sb = wpool.tile([128, KS * KS * C_OUT], f32r)
            nc.sync.dma_start(out=w_sb[:], in_=w_d[:])
            bias_sb = wpool.tile([128, n_half], f32)
            nc.sync.dma_start(out=bias_sb[:], in_=b_d[:])

            loop_cm = (
                tc.For_i(0, loop_n, 1, staggered_reset=loop_stagger)
                if loop_n > 1
                else contextlib.nullcontext()
            )
            with loop_cm:
              for b in range(B_SH):
                x_sb = xpool.tile([128, HP * WP], f32r)
                x3 = x_sb[:].rearrange("c (h w) -> c h w", w=WP)
                # image arrives pre-padded (114x114 with zero border) from
                # the host, so this is one fully contiguous DMA (optionally
                # split into x_split pieces for finer prefetch overlap).
                if with_load:
                    xd_flat = x_d[b].rearrange("c h w -> c (h w)")
                    step = HP * WP // x_split
                    for s in range(x_split):
                        lo = s * step
                        hi = HP * WP if s == x_split - 1 else (s + 1) * step
                        nc.sync.dma_start(
                            out=x_sb[:, lo:hi], in_=xd_flat[:, lo:hi]
                        )
                use_taps = taps if taps_subset is None else taps[:taps_subset]
                G = chunk_group
                SB = store_batch
                assert (H // R) % (SB * G) == 0 or SB == 1
                osb = {}
                for h0 in range(0, H, R * G):
                    # G row-chunks x n_half psum banks in flight; tap-inner
                    # ordering reuses each stationary weight tile G times.
                    pss = {}
                    for g in range(G):
                        for m in range(n_half):
                            pss[(g, m)] = psum_pool.tile(
                                [128, R * W], f32, name="ps", tag="ps"
                            )
                    if with_compute:
                        for m in range(n_half):
                            for i, (dh, dw) in enumerate(use_taps):
                                t = (dh + 1) * KS + (dw + 1)
                                co = t * C_OUT + m * 128
                                for g in range(G):
                                    hg = h0 + g * R
                                    nc.tensor.matmul(
                                        pss[(g, m)][:].rearrange(
                                            "p (r w) -> p r w", w=W
                                        ),
                                        w_sb[:, co : co + 128],
                                        x3[:, hg + dh + 1 : hg + dh + 1 + R,
                                           dw + 1 : dw + 1 + W],
                                        start=(i == 0),
                                        stop=(i == len(use_taps) - 1),
                                    )
                    if with_store:
                        for g in range(G):
                            for m in range(n_half):
                                hg = h0 + g * R
                                ps = pss[(g, m)]
                                j = (hg // R) % SB
                                if j == 0:
                                    osb[m] = opool.tile(
                                        [128, SB * R * W], f32,
                                        name="osb", tag="osb",
                                    )
                                dst = osb[m][:, j * R * W : (j + 1) * R * W]
                                if with_compute:
                                    nc.vector.tensor_scalar_add(
                                        dst, ps[:], bias_sb[:, m : m + 1]
                                    )
                                else:
                                    nc.vector.tensor_scalar_add(
                                        dst,
                                        x_sb[:, : R * W].bitcast(f32),
                                        bias_sb[:, m : m + 1],
                                    )
                                if j == SB - 1:
                                    nc.sync.dma_start(
                                        out=out_d[
                                            b, m * 128 : (m + 1) * 128,
                                            hg - (SB - 1) * R : hg + R, :,
                                        ],
                                        in_=osb[m][:],
                                    )
    nc.finalize()
    return nc


def _get_nc():
    if "nc" not in _cache:
        _cache["nc"] = _build()
    return _cache["nc"]


def _make_fast_runner(nc):
    """Cached jitted shard_map callable (axon/PJRT only). Avoids the ~20s
    per-call retrace that run_bass_kernel_spmd pays. Returns
    run(in_maps) -> full-batch output array."""
    import jax
    import numpy as _np
    from jax.sharding import Mesh, NamedSharding, PartitionSpec
    from jax.experimental.shard_map import shard_map
    import concourse.mybir as mybir
    from concourse import bass2jax as b2j

    b2j.install_neuronx_cc_hook()
    partition_name = nc.partition_id_tensor.name if nc.partition_id_tensor else None
    in_names, out_names, out_avals = [], [], []
    for alloc in nc.m.functions[0].allocations:
        if not isinstance(alloc, mybir.MemoryLocationSet):
            continue
        name = alloc.memorylocations[0].name
        if alloc.kind == "ExternalInput":
            if name != partition_name:
                in_names.append(name)
        elif alloc.kind == "ExternalOutput":
            out_names.append(name)
            out_avals.append(
                jax.core.ShapedArray(
                    tuple(alloc.tensor_shape), mybir.dt.np(alloc.dtype)
                )
            )
    assert out_names == ["out"]
    n_params = len(in_names)
    all_in_names = list(in_names) + list(out_names)
    if partition_name is not None:
        all_in_names.append(partition_name)

    def _body(*args):
        operands = list(args)
        if partition_name is not None:
            operands.append(b2j.partition_id_tensor())
        return tuple(
            b2j._bass_exec_p.bind(
                *operands,
                out_avals=tuple(out_avals),
                in_names=tuple(all_in_names),
                out_names=tuple(out_names),
                lowering_input_output_aliases=(),
                sim_require_finite=True,
                sim_require_nnan=True,
                nc=nc,
            )
        )

    devices = jax.devices()[:N_CORES]
    assert len(devices) == N_CORES
    mesh = Mesh(_np.asarray(devices), ("core",))
    n_outs = len(out_avals)
    fn = jax.jit(
        shard_map(
            _body,
            mesh=mesh,
            in_specs=(PartitionSpec("core"),) * (n_params + n_outs),
            out_specs=(PartitionSpec("core"),) * n_outs,
            check_rep=False,
        ),
        keep_unused=True,
    )
    shard = NamedSharding(mesh, PartitionSpec("core"))
    zeros = [
        jax.device_put(
            _np.zeros((N_CORES * av.shape[0], *av.shape[1:]), av.dtype), shard
        )
        for av in out_avals
    ]

    def run(in_maps):
        ins = [
            jax.device_put(
                _np.concatenate([m[name] for m in in_maps], axis=0), shard
            )
            for name in in_names
        ]
        outs = fn(*ins, *zeros)
        return _np.asarray(outs[0]).reshape(B, C_OUT, H, W)

    return run


def _install_neff_cache():
    """Content-addressed NEFF cache keyed on the BIR bytes. The bass_exec
    compile path (bass2jax.neuronx_cc_hook -> compile_bir_kernel) has no
    caching, so every fresh process pays the full walrus compile (30s-6min)
    for a byte-identical NEFF. The Bass build is deterministic, so caching
    on sha256(ant_bir) is sound. Best effort: any failure falls through to
    the original compile."""
    try:
        import hashlib
        import pathlib
        import shutil

        from concourse import bass2jax as b2j

        if getattr(b2j, "_ant_neff_cache_installed", False):
            return
        orig = b2j.compile_bir_kernel
        cache_dir = pathlib.Path.home() / ".cache" / "bass_neff_cache"
        cache_dir.mkdir(parents=True, exist_ok=True)

        def cached(ant_bir_str, compile_dir_path, neff_name="file.neff", **kw):
            import os

            try:
                raw = (
                    ant_bir_str
                    if isinstance(ant_bir_str, bytes)
                    else str(ant_bir_str).encode()
                )
                key = hashlib.sha256(raw).hexdigest()
                cpath = cache_dir / f"{key}_{neff_name}"
                if cpath.exists():
                    out = os.path.join(compile_dir_path, neff_name)
                    shutil.copyfile(cpath, out)
                    return out
            except Exception:
                cpath = None
            res = orig(ant_bir_str, compile_dir_path, neff_name=neff_name, **kw)
            if cpath is not None:
                try:
                    tmp = str(cpath) + ".tmp"
                    shutil.copyfile(res, tmp)
                    os.replace(tmp, cpath)
                except Exception:
                    pass
            return res

        b2j.compile_bir_kernel = cached
        b2j._ant_neff_cache_installed = True
    except Exception:
        pass


def _run_spmd(in_maps):
    from concourse.bass_utils import run_bass_kernel_spmd

    res = run_bass_kernel_spmd(_get_nc(), in_maps, core_ids=list(range(N_CORES)))
    return np.concatenate(
        [res.results[c]["out"] for c in range(N_CORES)], axis=0
    )


def _run(in_maps):
    if "runner" not in _cache:
        _install_neff_cache()
        runner = None
        try:
            from concourse._compat import axon_active

            if axon_active():
                runner = _make_fast_runner(_get_nc())
        except Exception:
            runner = None
        _cache["runner"] = runner if runner is not None else _run_spmd
    try:
        return _cache["runner"](in_maps)
    except Exception:
        if _cache["runner"] is _run_spmd:
            raise
        # fast path failed at call time: fall back permanently
        _cache["runner"] = _run_spmd
        return _run_spmd(in_maps)


def kernel(x, weight, bias, stride=1, padding=1, **_ignored):
    assert int(stride) == 1 and int(padding) == 1
    x = np.asarray(x, dtype=np.float32)
    weight = np.asarray(weight, dtype=np.float32)
    bias = np.asarray(bias, dtype=np.float32)
    assert x.shape == (B, C_IN, H, W) and weight.shape == (C_OUT, C_IN, KS, KS)
    xp = np.pad(x, ((0, 0), (0, 0), (1, 1), (1, 1)))

    # [c_out, c_in, kh, kw] -> [c_in, kh, kw, c_out] so that the lhsT slice
    # for (tap, half) is contiguous along c_out with c_in on partitions.
    w_t = np.ascontiguousarray(np.transpose(weight, (1, 2, 3, 0))).reshape(
        C_IN, KS * KS * C_OUT
    )
    bias2 = np.ascontiguousarray(bias.reshape(C_OUT // 128, 128).T)

    in_maps = [
        {
            "x": np.ascontiguousarray(xp[c * B_SH : (c + 1) * B_SH]),
            "w": w_t,
            "bias": bias2,
        }
        for c in range(N_CORES)
    ]
    return _run(in_maps)



# revision 14
# speedup vs baseline: 51866.4982x; 1.1508x over previous
"""Trainium2 Bass kernel for CustomConv2d:
x[32,128,112,112] (f32) * weight[256,128,3,3] + bias[256], stride=1, pad=1
-> out[32,256,112,112] (f32).

Strategy: data-parallel over batch (4 images per core on 8 cores). On each
core the conv is computed as 9 shift-accumulated fp32r matmuls per output
tile: contraction dim = C_IN = 128 (exactly the PE array), weights
stationary [c_in=128, c_out_half=128], moving operand = 4 output rows
(448 spatial positions) of the input image resident in SBUF. The image is
stored zero-padded to 114x114 per partition so every tap is a full-size
matmul whose APs satisfy the fp32r ISA restrictions (even innermost
counts, even outer steps). Weights are pre-transposed on the host to
[c_in, kh, kw, c_out] so every weight DMA and matmul slice is contiguous.

fp32r runs the PE at full rate (1 cycle/row for N>=256, vs 4 for fp32)
with ~1.4e-4 relative error vs the fp32 reference. Per-core roofline:
2016 matmuls x 448 cycles; compute-only (no I/O DMA) measures ~313 us,
full kernel ~390-410 us (in-NEFF loop slope, interleaved A/B).

Tuning findings (all interleaved A/B on-device, drift-cancelled):
- staggered For_i reset (loop_stagger=True) removes the all-engine
  barrier between timing-loop iterations: ~-40 us/pass.
- o_bufs 8->16 and x_bufs 2->3: ~-15 us/pass combined (deeper
  store-staging and image-prefetch slack).
- REGRESSIONS (rejected): bf16 matmul operands (+50 us), bf16 output
  staging (+58 us; DVE PSUM->bf16 drain is slower and store DMA was
  never the bottleneck), chunk_group>1 weight reuse (+80-120 us),
  x_split / store_batch changes (neutral).
"""

import numpy as np

B, C_IN, H, W = 32, 128, 112, 112
C_OUT, KS = 256, 3
N_CORES = 8
B_SH = B // N_CORES  # images per core
R = 4                # output rows per PSUM tile -> N = R*W = 448 <= 512
HP, WP = H + 2, W + 2  # padded image dims (114 x 114)

_cache = {}

# Tuned configuration (shared by kernel() and test.py's timed builds):
# bf16 input/output DMA halves HBM traffic (matmuls stay fp32r);
# staggered For_i reset avoids the all-engine barrier between timing-loop
# iterations.
IN_BF16 = False  # bf16 x + on-device widen: walrus codegen rejects the
                 # DVE bitcast copy, and halved store DMA showed no gain
                 # anyway (not bandwidth-bound) — keep f32r input DMA.
OUT_BF16 = False  # interleaved A/B: bf16 output staging REGRESSES ~58us
                  # (DVE PSUM->bf16 drain is slower; store DMA was never
                  # the bottleneck) — keep f32 output.
LOOP_STAGGER = True  # staggered For_i reset: removes the all-engine
                     # barrier between loop iterations, ~40us/pass
                     # (interleaved A/B: 446.7 -> 408.1 us).


def _build(psum_bufs=8, o_bufs=16, x_bufs=3, with_load=True, with_compute=True,
           with_store=True, x_split=28, taps_subset=None, loop_n=1,
           chunk_group=1, loop_stagger=None, store_batch=1, dtype="f32r",
           out_dtype=None, in_bf16=None):
    if loop_stagger is None:
        loop_stagger = LOOP_STAGGER
    if out_dtype is None:
        out_dtype = "bf16" if OUT_BF16 else "f32"
    if in_bf16 is None:
        in_bf16 = IN_BF16
    import contextlib

    import concourse.mybir as mybir
    from concourse import bacc
    from concourse.tile import TileContext

    f32 = mybir.dt.float32
    f32r = mybir.dt.float32r if dtype == "f32r" else mybir.dt.bfloat16
    f32o = mybir.dt.float32 if out_dtype == "f32" else mybir.dt.bfloat16
    bf16 = mybir.dt.bfloat16
    x_load_dt = bf16 if in_bf16 else f32r

    nc = bacc.Bacc("TRN2")
    x_d = nc.dram_tensor("x", [B_SH, C_IN, HP, WP], x_load_dt, kind="ExternalInput")
    w_d = nc.dram_tensor("w", [C_IN, KS * KS * C_OUT], f32r, kind="ExternalInput")
    b_d = nc.dram_tensor("bias", [128, C_OUT // 128], f32, kind="ExternalInput")
    out_d = nc.dram_tensor("out", [B_SH, C_OUT, H, W], f32o, kind="ExternalOutput")

    n_half = C_OUT // 128  # 2
    taps = [(dh, dw) for dh in (-1, 0, 1) for dw in (-1, 0, 1)]

    with TileContext(nc) as tc:
        with (
            tc.tile_pool(name="wpool", bufs=1) as wpool,
            tc.tile_pool(name="xpool", bufs=x_bufs) as xpool,
            tc.tile_pool(name="xbpool", bufs=2) as xbpool,
            tc.tile_pool(name="opool", bufs=o_bufs) as opool,
            tc.tile_pool(name="psum", bufs=psum_bufs, space="PSUM") as psum_pool,
        ):
            w_sb = wpool.tile([128, KS * KS * C_OUT], f32r)
            nc.sync.dma_start(out=w_sb[:], in_=w_d[:])
            bias_sb = wpool.tile([128, n_half], f32)
            nc.sync.dma_start(out=bias_sb[:], in_=b_d[:])

            loop_cm = (
                tc.For_i(0, loop_n, 1, staggered_reset=loop_stagger)
                if loop_n > 1
                else contextlib.nullcontext()
            )
            with loop_cm:
              for b in range(B_SH):
                x_sb = xpool.tile([128, HP * WP], f32r)
                x3 = x_sb[:].rearrange("c (h w) -> c h w", w=WP)
                # image arrives pre-padded (114x114 with zero border) from
                # the host, so this is one fully contiguous DMA (optionally
                # split into x_split pieces for finer prefetch overlap).
                # in_bf16: DMA the bf16 image into a staging tile and widen
                # to f32 on DVE per piece (halves input HBM traffic; the
                # matmuls stay fp32r).
                xd_flat = x_d[b].rearrange("c h w -> c (h w)")
                step = HP * WP // x_split
                if in_bf16:
                    xb_sb = xbpool.tile([128, HP * WP], bf16)
                # with_load=False still writes piece 0 so the tile framework
                # sees the tile initialized (timing-differential builds only).
                for s in range(x_split if with_load else 1):
                    lo = s * step
                    hi = HP * WP if s == x_split - 1 else (s + 1) * step
                    if in_bf16:
                        nc.sync.dma_start(
                            out=xb_sb[:, lo:hi], in_=xd_flat[:, lo:hi]
                        )
                        nc.vector.tensor_copy(
                            x_sb[:, lo:hi].bitcast(f32), xb_sb[:, lo:hi]
                        )
                    else:
                        nc.sync.dma_start(
                            out=x_sb[:, lo:hi], in_=xd_flat[:, lo:hi]
                        )
                use_taps = taps if taps_subset is None else taps[:taps_subset]
                G = chunk_group
                SB = store_batch
                assert (H // R) % (SB * G) == 0 or SB == 1
                osb = {}
                for h0 in range(0, H, R * G):
                    # G row-chunks x n_half psum banks in flight; tap-inner
                    # ordering reuses each stationary weight tile G times.
                    pss = {}
                    for g in range(G):
                        for m in range(n_half):
                            pss[(g, m)] = psum_pool.tile(
                                [128, R * W], f32, name="ps", tag="ps"
                            )
                    if with_compute:
                        for m in range(n_half):
                            for i, (dh, dw) in enumerate(use_taps):
                                t = (dh + 1) * KS + (dw + 1)
                                co = t * C_OUT + m * 128
                                for g in range(G):
                                    hg = h0 + g * R
                                    nc.tensor.matmul(
                                        pss[(g, m)][:].rearrange(
                                            "p (r w) -> p r w", w=W
                                        ),
                                        w_sb[:, co : co + 128],
                                        x3[:, hg + dh + 1 : hg + dh + 1 + R,
                                           dw + 1 : dw + 1 + W],
                                        start=(i == 0),
                                        stop=(i == len(use_taps) - 1),
                                    )
                    if with_store:
                        for g in range(G):
                            for m in range(n_half):
                                hg = h0 + g * R
                                ps = pss[(g, m)]
                                j = (hg // R) % SB
                                if j == 0:
                                    osb[m] = opool.tile(
                                        [128, SB * R * W], f32o,
                                        name="osb", tag="osb",
                                    )
                                dst = osb[m][:, j * R * W : (j + 1) * R * W]
                                if with_compute:
                                    nc.vector.tensor_scalar_add(
                                        dst, ps[:], bias_sb[:, m : m + 1]
                                    )
                                else:
                                    nc.vector.tensor_scalar_add(
                                        dst,
                                        x_sb[:, : R * W].bitcast(f32),
                                        bias_sb[:, m : m + 1],
                                    )
                                if j == SB - 1:
                                    nc.sync.dma_start(
                                        out=out_d[
                                            b, m * 128 : (m + 1) * 128,
                                            hg - (SB - 1) * R : hg + R, :,
                                        ],
                                        in_=osb[m][:],
                                    )
    nc.finalize()
    return nc


def _get_nc():
    if "nc" not in _cache:
        _cache["nc"] = _build()
    return _cache["nc"]


def _make_fast_runner(nc):
    """Cached jitted shard_map callable (axon/PJRT only). Avoids the ~20s
    per-call retrace that run_bass_kernel_spmd pays. Returns
    run(in_maps) -> full-batch output array."""
    import jax
    import numpy as _np
    from jax.sharding import Mesh, NamedSharding, PartitionSpec
    from jax.experimental.shard_map import shard_map
    import concourse.mybir as mybir
    from concourse import bass2jax as b2j

    b2j.install_neuronx_cc_hook()
    partition_name = nc.partition_id_tensor.name if nc.partition_id_tensor else None
    in_names, out_names, out_avals = [], [], []
    for alloc in nc.m.functions[0].allocations:
        if not isinstance(alloc, mybir.MemoryLocationSet):
            continue
        name = alloc.memorylocations[0].name
        if alloc.kind == "ExternalInput":
            if name != partition_name:
                in_names.append(name)
        elif alloc.kind == "ExternalOutput":
            out_names.append(name)
            out_avals.append(
                jax.core.ShapedArray(
                    tuple(alloc.tensor_shape), mybir.dt.np(alloc.dtype)
                )
            )
    assert out_names == ["out"]
    n_params = len(in_names)
    all_in_names = list(in_names) + list(out_names)
    if partition_name is not None:
        all_in_names.append(partition_name)

    def _body(*args):
        operands = list(args)
        if partition_name is not None:
            operands.append(b2j.partition_id_tensor())
        return tuple(
            b2j._bass_exec_p.bind(
                *operands,
                out_avals=tuple(out_avals),
                in_names=tuple(all_in_names),
                out_names=tuple(out_names),
                lowering_input_output_aliases=(),
                sim_require_finite=True,
                sim_require_nnan=True,
                nc=nc,
            )
        )

    devices = jax.devices()[:N_CORES]
    assert len(devices) == N_CORES
    mesh = Mesh(_np.asarray(devices), ("core",))
    n_outs = len(out_avals)
    fn = jax.jit(
        shard_map(
            _body,
            mesh=mesh,
            in_specs=(PartitionSpec("core"),) * (n_params + n_outs),
            out_specs=(PartitionSpec("core"),) * n_outs,
            check_rep=False,
        ),
        keep_unused=True,
    )
    shard = NamedSharding(mesh, PartitionSpec("core"))
    zeros = [
        jax.device_put(
            _np.zeros((N_CORES * av.shape[0], *av.shape[1:]), av.dtype), shard
        )
        for av in out_avals
    ]

    def run(in_maps):
        ins = [
            jax.device_put(
                _np.concatenate([m[name] for m in in_maps], axis=0), shard
            )
            for name in in_names
        ]
        outs = fn(*ins, *zeros)
        return _np.asarray(outs[0]).reshape(B, C_OUT, H, W)

    return run


def _install_neff_cache():
    """Content-addressed NEFF cache keyed on the BIR bytes. The bass_exec
    compile path (bass2jax.neuronx_cc_hook -> compile_bir_kernel) has no
    caching, so every fresh process pays the full walrus compile (30s-6min)
    for a byte-identical NEFF. The Bass build is deterministic, so caching
    on sha256(ant_bir) is sound. Best effort: any failure falls through to
    the original compile."""
    try:
        import hashlib
        import pathlib
        import shutil

        from concourse import bass2jax as b2j

        if getattr(b2j, "_ant_neff_cache_installed", False):
            return
        orig = b2j.compile_bir_kernel
        cache_dir = pathlib.Path.home() / ".cache" / "bass_neff_cache"
        cache_dir.mkdir(parents=True, exist_ok=True)

        def cached(ant_bir_str, compile_dir_path, neff_name="file.neff", **kw):
            import os

            try:
                raw = (
                    ant_bir_str
                    if isinstance(ant_bir_str, bytes)
                    else str(ant_bir_str).encode()
                )
                key = hashlib.sha256(raw).hexdigest()
                cpath = cache_dir / f"{key}_{neff_name}"
                if cpath.exists():
                    out = os.path.join(compile_dir_path, neff_name)
                    shutil.copyfile(cpath, out)
                    return out
            except Exception:
                cpath = None
            res = orig(ant_bir_str, compile_dir_path, neff_name=neff_name, **kw)
            if cpath is not None:
                try:
                    tmp = str(cpath) + ".tmp"
                    shutil.copyfile(res, tmp)
                    os.replace(tmp, cpath)
                except Exception:
                    pass
            return res

        b2j.compile_bir_kernel = cached
        b2j._ant_neff_cache_installed = True
    except Exception:
        pass


def _run_spmd(in_maps):
    from concourse.bass_utils import run_bass_kernel_spmd

    res = run_bass_kernel_spmd(_get_nc(), in_maps, core_ids=list(range(N_CORES)))
    return np.concatenate(
        [res.results[c]["out"] for c in range(N_CORES)], axis=0
    )


def _run(in_maps):
    if "runner" not in _cache:
        _install_neff_cache()
        runner = None
        try:
            from concourse._compat import axon_active

            if axon_active():
                runner = _make_fast_runner(_get_nc())
        except Exception:
            runner = None
        _cache["runner"] = runner if runner is not None else _run_spmd
    try:
        return _cache["runner"](in_maps)
    except Exception:
        if _cache["runner"] is _run_spmd:
            raise
        # fast path failed at call time: fall back permanently
        _cache["runner"] = _run_spmd
        return _run_spmd(in_maps)


def prep_in_maps(x, weight, bias):
    """Host-side prep matching _build()'s default dram layout/dtypes."""
    x = np.asarray(x, dtype=np.float32)
    weight = np.asarray(weight, dtype=np.float32)
    bias = np.asarray(bias, dtype=np.float32)
    assert x.shape == (B, C_IN, H, W) and weight.shape == (C_OUT, C_IN, KS, KS)
    xp = np.pad(x, ((0, 0), (0, 0), (1, 1), (1, 1)))
    if IN_BF16:
        import ml_dtypes

        xp = xp.astype(ml_dtypes.bfloat16)

    # [c_out, c_in, kh, kw] -> [c_in, kh, kw, c_out] so that the lhsT slice
    # for (tap, half) is contiguous along c_out with c_in on partitions.
    w_t = np.ascontiguousarray(np.transpose(weight, (1, 2, 3, 0))).reshape(
        C_IN, KS * KS * C_OUT
    )
    bias2 = np.ascontiguousarray(bias.reshape(C_OUT // 128, 128).T)

    return [
        {
            "x": np.ascontiguousarray(xp[c * B_SH : (c + 1) * B_SH]),
            "w": w_t,
            "bias": bias2,
        }
        for c in range(N_CORES)
    ]


def kernel(x, weight, bias, stride=1, padding=1, **_ignored):
    assert int(stride) == 1 and int(padding) == 1
    out = _run(prep_in_maps(x, weight, bias))
    return np.ascontiguousarray(out.astype(np.float32, copy=False))

